# revision 14
# baseline (speedup 1.0000x reference)
"""Trainium2 Bass kernel for CoocOpModel.

out[b,s,z] = sum_{i,j} func[b,s,i] * cooc[i,j,z] * arg[b,s,j]
  with func = func_and_arg[..., :128], arg = func_and_arg[..., 128:]

Shapes (hardcoded): func_and_arg [4,1024,256] f32, cooccurrences [128,128,128] f32,
out [4,1024,128] f32.  D = 128, tokens T = 4096.

Strategy: data-parallel over tokens across 8 cores (512 tokens/core);
cooccurrence tensor replicated per core (fp16).

Per-core math, with t = local token index (512), i/j/z in [0,128):
  out_T[z, t] = sum_i  C_i^T @ G_i        (accumulated in one PSUM bank)
  C_i[j, z]   = cooc[i, j, z]             (stationary operand, fp16)
  G_i[j, t]   = arg_T[j, t] * func_T[i, t]  (moving operand, fp16)

The f_exp broadcast (func_T rows replicated across the 128 j-partitions,
16 MB/core) dominates DMA; the two HWDGE queues stream it at the ~410 GB/s
port limit, which set the old ~72 us floor. v4 moves 6 of the 16 i-groups
off DMA: a K=4 selector matmul (sel column picks one of 4 staged func rows)
replicates a row into PSUM ([128,512] broadcast, exact), and the scalar
engine drains 3 banks per ACTIVATE (amortizing its 352-cycle overhead) into
the f_exp tile as fp16. Broadcast matmuls and drains are interleaved in
small doses between main-group matmuls to avoid engine-FIFO head-of-line
blocking.
"""

import sys

sys.path.insert(0, "/opt/trn_rl_repo")

import numpy as np
from contextlib import ExitStack

import concourse.bass as bass
import concourse.tile as tile
from concourse import bacc, mybir
from concourse.bass_utils import run_bass_kernel_spmd

F16 = mybir.dt.float16
F32 = mybir.dt.float32
NP_F16 = np.float16

N_CORES = 8
D = 128
T_TOTAL = 4096
T_CORE = T_TOTAL // N_CORES  # 512

# group sizes over the i axis; small head groups so compute ramps early,
# small tail group so the post-DMA serial tail is short.
SIZES = [2, 2, 4, 4] + [8] * 14 + [4]
assert sum(SIZES) == D
N_GRP = len(SIZES)
STARTS = np.cumsum([0] + SIZES).tolist()

# compute order: the last broadcast group runs after the DMA-fed tail so
# its ACT drains are never the critical path at the end of the kernel
ORDER = list(range(15)) + [16, 17, 18, 15]
POS_OF = {g: p for p, g in enumerate(ORDER)}

# groups whose f_exp comes from the PE selector-broadcast instead of DMA
BCAST_GROUPS = (5, 7, 9, 11, 13, 15)
BCAST_IS = [i for g in BCAST_GROUPS for i in range(STARTS[g], STARTS[g] + SIZES[g])]
NB = len(BCAST_IS)
NB_SLOTS = (NB + 3) // 4  # f4 slots per staging partition

_NC_CACHE = None


def _build():
    nc = bacc.Bacc("TRN2", target_bir_lowering=False, debug=False, num_devices=N_CORES)

    f_t = nc.dram_tensor("f_t", [D, T_CORE], F16, kind="ExternalInput").ap()
    a_t = nc.dram_tensor("a_t", [D, T_CORE], F16, kind="ExternalInput").ap()
    # c2[j, i*128 + z] = cooc[i, j, z]
    c2 = nc.dram_tensor("c2", [D, D * D], F16, kind="ExternalInput").ap()
    # f4[q, slot*512 + t] = func_T row for broadcast index (see _prep)
    f4 = nc.dram_tensor("f4", [4, NB_SLOTS * T_CORE], F16, kind="ExternalInput").ap()
    # sel[k, q*128 + j] = (k == q): K=4 selector columns for the broadcast matmul
    sel = nc.dram_tensor("sel", [4, 4 * D], F16, kind="ExternalInput").ap()
    out_t = nc.dram_tensor("out_t", [D, T_CORE], F16, kind="ExternalOutput").ap()

    with tile.TileContext(nc) as tc:
        with ExitStack() as ctx:
            const_pool = ctx.enter_context(tc.tile_pool(name="const", bufs=1))
            g_pool = ctx.enter_context(tc.tile_pool(name="g", bufs=3))
            out_pool = ctx.enter_context(tc.tile_pool(name="out", bufs=1))
            psum_pool = ctx.enter_context(
                tc.tile_pool(name="psum", bufs=1, space="PSUM")
            )
            psb_pool = ctx.enter_context(
                tc.tile_pool(name="psb", bufs=3, space="PSUM")
            )

            # ---- tiles
            a_sb = const_pool.tile([D, T_CORE], F16, tag="a")
            f4_sb = const_pool.tile([4, NB_SLOTS * T_CORE], F16, tag="f4")
            sel_sb = const_pool.tile([4, 4 * D], F16, tag="sel")
            f_tiles = []
            c_tiles = []
            for g, sz in enumerate(SIZES):
                f_tiles.append(
                    const_pool.tile([D, sz * T_CORE], F16, tag=f"fexp{g}", name=f"fexp{g}")
                )
                c_tiles.append(
                    const_pool.tile([D, sz * D], F16, tag=f"c{g}", name=f"c{g}")
                )

            # ---- DMA issue. scalar: head-critical pieces; sync: the rest.
            # f_exp transfers alternate queues to balance ring bytes.
            nc.scalar.dma_start(f4_sb[:], f4[:, :])
            nc.scalar.dma_start(sel_sb[:], sel[:, :])
            nc.sync.dma_start(a_sb[:], a_t[:, :])

            # all stream transfers issue from sync so the scalar engine's
            # FIFO holds nothing but f4/sel and the PSUM drains (a DMA issue
            # can block on completion-lane availability and would head-of-
            # line-block the drains behind it)
            for g in range(N_GRP):
                i0, sz = STARTS[g], SIZES[g]
                if g not in BCAST_GROUPS:
                    f_src = bass.AP(
                        f_t.tensor, i0 * T_CORE, [[0, D], [T_CORE, sz], [1, T_CORE]]
                    )
                    nc.sync.dma_start(f_tiles[g][:], f_src)
                nc.sync.dma_start(c_tiles[g][:], c2[:, i0 * D : (i0 + sz) * D])

            # ---- broadcast work: (group, chunk_start, chunk_len) items,
            # drained 3-2-3 banks per ACTIVATE. Emitted interleaved below.
            bcast_idx = {}
            for idx, i in enumerate(BCAST_IS):
                bcast_idx[i] = (idx % 4, idx // 4)

            bc_items = []
            for g in BCAST_GROUPS:
                i0, sz = STARTS[g], SIZES[g]
                k = 0
                while k < sz:
                    bc_items.append((g, k, 2))
                    k += 2
                assert k == sz

            bc_pos = 0

            def emit_bcast(n_items):
                nonlocal bc_pos
                for _ in range(n_items):
                    if bc_pos >= len(bc_items):
                        return
                    g, k0, w = bc_items[bc_pos]
                    bc_pos += 1
                    i0 = STARTS[g]
                    ps_b = psb_pool.tile([D, 2 * T_CORE], F32, tag="psb", name="psb")
                    for k in range(k0, k0 + w):
                        q, slot = bcast_idx[i0 + k]
                        nc.tensor.matmul(
                            ps_b[:, (k - k0) * T_CORE : (k - k0 + 1) * T_CORE],
                            sel_sb[:, q * D : (q + 1) * D],
                            f4_sb[:, slot * T_CORE : (slot + 1) * T_CORE],
                            start=True,
                            stop=True,
                        )
                    nc.scalar.copy(
                        f_tiles[g][:, k0 * T_CORE : (k0 + w) * T_CORE],
                        ps_b[:, : w * T_CORE],
                    )

            # ---- compute pipeline
            a_ap = a_sb[:]
            ps = psum_pool.tile([D, T_CORE], F32)
            n_done = 0
            for pos, g in enumerate(ORDER):
                sz = SIZES[g]
                i0 = STARTS[g]
                f_exp = f_tiles[g]
                c_sb = c_tiles[g]

                # broadcast producer first: its matmuls are ready (f4 is
                # resident) and fill the PE while the TT below still waits
                n_em = 0
                while (
                    bc_pos < len(bc_items)
                    and n_em < 3
                    and POS_OF[bc_items[bc_pos][0]] <= pos + 6
                ):
                    emit_bcast(1)
                    n_em += 1

                gt = g_pool.tile([D, sz * T_CORE], F16, tag="g")
                if g == N_GRP - 1:
                    # split tail multiply so matmuls overlap the second half
                    h = sz // 2
                    a_half = bass.AP(
                        a_ap.tensor, a_ap.offset, [a_ap.ap[0], [0, h], [1, T_CORE]]
                    )
                    nc.vector.tensor_mul(
                        gt[:, : h * T_CORE], a_half, f_exp[:, : h * T_CORE]
                    )
                    nc.vector.tensor_mul(
                        gt[:, h * T_CORE :], a_half, f_exp[:, h * T_CORE :]
                    )
                else:
                    a_view = bass.AP(
                        a_ap.tensor, a_ap.offset, [a_ap.ap[0], [0, sz], [1, T_CORE]]
                    )
                    nc.vector.tensor_mul(gt[:], a_view, f_exp[:])

                for k in range(sz):
                    n_done += 1
                    nc.tensor.matmul(
                        ps[:],
                        c_sb[:, k * D : (k + 1) * D],
                        gt[:, k * T_CORE : (k + 1) * T_CORE],
                        start=(n_done == 1),
                        stop=(n_done == D),
                    )

            while bc_pos < len(bc_items):
                emit_bcast(1)

            o_sb = out_pool.tile([D, T_CORE], F16, tag="o")
            nc.scalar.copy(o_sb[:], ps[:])
            nc.sync.dma_start(out_t[:, :], o_sb[:])

    nc.compile()
    return nc


def _get_nc():
    global _NC_CACHE
    if _NC_CACHE is None:
        _NC_CACHE = _build()
    return _NC_CACHE


def _prep_in_maps(func_and_arg, cooccurrences):
    fa = np.asarray(func_and_arg, dtype=np.float32).reshape(T_TOTAL, 2 * D)
    c2 = (
        np.ascontiguousarray(
            np.asarray(cooccurrences, dtype=np.float32).transpose(1, 0, 2)
        )
        .reshape(D, D * D)
        .astype(NP_F16)
    )
    sel = np.zeros((4, 4 * D), dtype=NP_F16)
    for q in range(4):
        sel[q, q * D : (q + 1) * D] = 1.0
    in_maps = []
    for c in range(N_CORES):
        s = fa[c * T_CORE : (c + 1) * T_CORE]  # [512, 256]
        f_tc = np.ascontiguousarray(s[:, :D].T).astype(NP_F16)  # [128 i, 512 t]
        a_tc = np.ascontiguousarray(s[:, D:].T).astype(NP_F16)  # [128 j, 512 t]
        f4 = np.zeros((4, NB_SLOTS * T_CORE), dtype=NP_F16)
        for idx, i in enumerate(BCAST_IS):
            q, slot = idx % 4, idx // 4
            f4[q, slot * T_CORE : (slot + 1) * T_CORE] = f_tc[i]
        in_maps.append({"f_t": f_tc, "a_t": a_tc, "c2": c2, "f4": f4, "sel": sel})
    return in_maps


def kernel(func_and_arg: np.ndarray, cooccurrences: np.ndarray) -> np.ndarray:
    assert func_and_arg.shape == (4, 1024, 2 * D)
    assert cooccurrences.shape == (D, D, D)

    in_maps = _prep_in_maps(func_and_arg, cooccurrences)
    nc = _get_nc()
    res = run_bass_kernel_spmd(nc, in_maps, core_ids=list(range(N_CORES)))

    # out_t per core: [z=128, t=512] fp16 -> [t, z]; concat -> [4096, 128]
    outs = [res.results[c]["out_t"].T for c in range(N_CORES)]
    out = np.concatenate(outs, axis=0).reshape(4, 1024, D).astype(np.float32)
    return out


# revision 15
# speedup vs baseline: 1.2260x; 1.2260x over previous
"""Trainium2 Bass kernel for CoocOpModel.

out[b,s,z] = sum_{i,j} func[b,s,i] * cooc[i,j,z] * arg[b,s,j]
  with func = func_and_arg[..., :128], arg = func_and_arg[..., 128:]

Shapes (hardcoded): func_and_arg [4,1024,256] f32, cooccurrences [128,128,128] f32,
out [4,1024,128] f32.  D = 128, tokens T = 4096.

Strategy: data-parallel over tokens across 8 cores (512 tokens/core);
cooccurrence tensor replicated per core (fp16).

Per-core math, with t = local token index (512), i/j/z in [0,128):
  out_T[z, t] = sum_i  C_i^T @ G_i        (accumulated in one PSUM bank)
  C_i[j, z]   = cooc[i, j, z]             (stationary operand, fp16)
  G_i[j, t]   = arg_T[j, t] * func_T[i, t]  (moving operand, fp16)

The f_exp broadcast (func_T rows replicated across the 128 j-partitions,
16 MB/core) dominates DMA; the two HWDGE queues stream it at the ~410 GB/s
port limit, which set the old ~72 us floor. v4 moves 6 of the 16 i-groups
off DMA: a K=4 selector matmul (sel column picks one of 4 staged func rows)
replicates a row into PSUM ([128,512] broadcast, exact), and the scalar
engine drains 3 banks per ACTIVATE (amortizing its 352-cycle overhead) into
the f_exp tile as fp16. Broadcast matmuls and drains are interleaved in
small doses between main-group matmuls to avoid engine-FIFO head-of-line
blocking.
"""

import sys

sys.path.insert(0, "/opt/trn_rl_repo")

import numpy as np
from contextlib import ExitStack

import concourse.bass as bass
import concourse.tile as tile
from concourse import bacc, mybir
from concourse.bass_utils import run_bass_kernel_spmd

F16 = mybir.dt.float16
F32 = mybir.dt.float32
NP_F16 = np.float16

N_CORES = 8
D = 128
T_TOTAL = 4096
T_CORE = T_TOTAL // N_CORES  # 512

# group sizes over the i axis; small head groups so compute ramps early,
# small tail group so the post-DMA serial tail is short.
SIZES = [2, 2, 4, 4] + [8] * 14 + [4]
assert sum(SIZES) == D
N_GRP = len(SIZES)
STARTS = np.cumsum([0] + SIZES).tolist()

ORDER = list(range(N_GRP))
POS_OF = {g: p for p, g in enumerate(ORDER)}

# groups whose f_exp comes from the PE selector-broadcast instead of DMA
BCAST_GROUPS = (5, 7, 9, 11, 13)
BCAST_IS = [i for g in BCAST_GROUPS for i in range(STARTS[g], STARTS[g] + SIZES[g])]
NB = len(BCAST_IS)
NB_SLOTS = (NB + 3) // 4  # f4 slots per staging partition

_NC_CACHE = None


def _build():
    nc = bacc.Bacc("TRN2", target_bir_lowering=False, debug=False, num_devices=N_CORES)

    f_t = nc.dram_tensor("f_t", [D, T_CORE], F16, kind="ExternalInput").ap()
    a_t = nc.dram_tensor("a_t", [D, T_CORE], F16, kind="ExternalInput").ap()
    # c2[j, i*128 + z] = cooc[i, j, z]
    c2 = nc.dram_tensor("c2", [D, D * D], F16, kind="ExternalInput").ap()
    # f4[q, slot*512 + t] = func_T row for broadcast index (see _prep)
    f4 = nc.dram_tensor("f4", [4, NB_SLOTS * T_CORE], F16, kind="ExternalInput").ap()
    # sel[k, q*128 + j] = (k == q): K=4 selector columns for the broadcast matmul
    sel = nc.dram_tensor("sel", [4, 4 * D], F16, kind="ExternalInput").ap()
    out_t = nc.dram_tensor("out_t", [D, T_CORE], F16, kind="ExternalOutput").ap()

    with tile.TileContext(nc) as tc:
        with ExitStack() as ctx:
            const_pool = ctx.enter_context(tc.tile_pool(name="const", bufs=1))
            g_pool = ctx.enter_context(tc.tile_pool(name="g", bufs=3))
            out_pool = ctx.enter_context(tc.tile_pool(name="out", bufs=1))
            psum_pool = ctx.enter_context(
                tc.tile_pool(name="psum", bufs=1, space="PSUM")
            )
            psb_pool = ctx.enter_context(
                tc.tile_pool(name="psb", bufs=3, space="PSUM")
            )

            # ---- tiles
            a_sb = const_pool.tile([D, T_CORE], F16, tag="a")
            f4_sb = const_pool.tile([4, NB_SLOTS * T_CORE], F16, tag="f4")
            sel_sb = const_pool.tile([4, 4 * D], F16, tag="sel")
            f_tiles = []
            c_tiles = []
            for g, sz in enumerate(SIZES):
                f_tiles.append(
                    const_pool.tile([D, sz * T_CORE], F16, tag=f"fexp{g}", name=f"fexp{g}")
                )
                c_tiles.append(
                    const_pool.tile([D, sz * D], F16, tag=f"c{g}", name=f"c{g}")
                )

            # ---- DMA issue. scalar: head-critical pieces; sync: the rest.
            # f_exp transfers alternate queues to balance ring bytes.
            nc.scalar.dma_start(f4_sb[:], f4[:, :])
            nc.scalar.dma_start(sel_sb[:], sel[:, :])
            nc.sync.dma_start(a_sb[:], a_t[:, :])

            # all stream transfers issue from sync so the scalar engine's
            # FIFO holds nothing but f4/sel and the PSUM drains (a DMA issue
            # can block on completion-lane availability and would head-of-
            # line-block the drains behind it)
            for g in range(N_GRP):
                i0, sz = STARTS[g], SIZES[g]
                if g not in BCAST_GROUPS:
                    f_src = bass.AP(
                        f_t.tensor, i0 * T_CORE, [[0, D], [T_CORE, sz], [1, T_CORE]]
                    )
                    nc.sync.dma_start(f_tiles[g][:], f_src)
                nc.sync.dma_start(c_tiles[g][:], c2[:, i0 * D : (i0 + sz) * D])

            # ---- broadcast work: (group, chunk_start, chunk_len) items,
            # drained 3-2-3 banks per ACTIVATE. Emitted interleaved below.
            bcast_idx = {}
            for idx, i in enumerate(BCAST_IS):
                bcast_idx[i] = (idx % 4, idx // 4)

            bc_items = []
            for g in BCAST_GROUPS:
                i0, sz = STARTS[g], SIZES[g]
                k = 0
                while k < sz:
                    bc_items.append((g, k, 2))
                    k += 2
                assert k == sz

            bc_pos = 0

            def emit_bcast(n_items):
                nonlocal bc_pos
                for _ in range(n_items):
                    if bc_pos >= len(bc_items):
                        return
                    g, k0, w = bc_items[bc_pos]
                    bc_pos += 1
                    i0 = STARTS[g]
                    ps_b = psb_pool.tile([D, 2 * T_CORE], F32, tag="psb", name="psb")
                    for k in range(k0, k0 + w):
                        q, slot = bcast_idx[i0 + k]
                        nc.tensor.matmul(
                            ps_b[:, (k - k0) * T_CORE : (k - k0 + 1) * T_CORE],
                            sel_sb[:, q * D : (q + 1) * D],
                            f4_sb[:, slot * T_CORE : (slot + 1) * T_CORE],
                            start=True,
                            stop=True,
                        )
                    nc.scalar.copy(
                        f_tiles[g][:, k0 * T_CORE : (k0 + w) * T_CORE],
                        ps_b[:, : w * T_CORE],
                    )

            # ---- compute pipeline
            a_ap = a_sb[:]
            ps = psum_pool.tile([D, T_CORE], F32)
            n_done = 0
            for pos, g in enumerate(ORDER):
                sz = SIZES[g]
                i0 = STARTS[g]
                f_exp = f_tiles[g]
                c_sb = c_tiles[g]

                # broadcast producer first: its matmuls are ready (f4 is
                # resident) and fill the PE while the TT below still waits
                n_em = 0
                while (
                    bc_pos < len(bc_items)
                    and n_em < 3
                    and POS_OF[bc_items[bc_pos][0]] <= pos + 4
                ):
                    emit_bcast(1)
                    n_em += 1

                gt = g_pool.tile([D, sz * T_CORE], F16, tag="g")
                if g == N_GRP - 1:
                    # split tail multiply so matmuls overlap the second half
                    h = sz // 2
                    a_half = bass.AP(
                        a_ap.tensor, a_ap.offset, [a_ap.ap[0], [0, h], [1, T_CORE]]
                    )
                    nc.vector.tensor_mul(
                        gt[:, : h * T_CORE], a_half, f_exp[:, : h * T_CORE]
                    )
                    nc.vector.tensor_mul(
                        gt[:, h * T_CORE :], a_half, f_exp[:, h * T_CORE :]
                    )
                else:
                    a_view = bass.AP(
                        a_ap.tensor, a_ap.offset, [a_ap.ap[0], [0, sz], [1, T_CORE]]
                    )
                    nc.vector.tensor_mul(gt[:], a_view, f_exp[:])

                for k in range(sz):
                    n_done += 1
                    nc.tensor.matmul(
                        ps[:],
                        c_sb[:, k * D : (k + 1) * D],
                        gt[:, k * T_CORE : (k + 1) * T_CORE],
                        start=(n_done == 1),
                        stop=(n_done == D),
                    )

            while bc_pos < len(bc_items):
                emit_bcast(1)

            o_sb = out_pool.tile([D, T_CORE], F16, tag="o")
            nc.scalar.copy(o_sb[:], ps[:])
            nc.sync.dma_start(out_t[:, :], o_sb[:])

    nc.compile()
    return nc


def _get_nc():
    global _NC_CACHE
    if _NC_CACHE is None:
        _NC_CACHE = _build()
    return _NC_CACHE


def _prep_in_maps(func_and_arg, cooccurrences):
    fa = np.asarray(func_and_arg, dtype=np.float32).reshape(T_TOTAL, 2 * D)
    c2 = (
        np.ascontiguousarray(
            np.asarray(cooccurrences, dtype=np.float32).transpose(1, 0, 2)
        )
        .reshape(D, D * D)
        .astype(NP_F16)
    )
    sel = np.zeros((4, 4 * D), dtype=NP_F16)
    for q in range(4):
        sel[q, q * D : (q + 1) * D] = 1.0
    in_maps = []
    for c in range(N_CORES):
        s = fa[c * T_CORE : (c + 1) * T_CORE]  # [512, 256]
        f_tc = np.ascontiguousarray(s[:, :D].T).astype(NP_F16)  # [128 i, 512 t]
        a_tc = np.ascontiguousarray(s[:, D:].T).astype(NP_F16)  # [128 j, 512 t]
        f4 = np.zeros((4, NB_SLOTS * T_CORE), dtype=NP_F16)
        for idx, i in enumerate(BCAST_IS):
            q, slot = idx % 4, idx // 4
            f4[q, slot * T_CORE : (slot + 1) * T_CORE] = f_tc[i]
        in_maps.append({"f_t": f_tc, "a_t": a_tc, "c2": c2, "f4": f4, "sel": sel})
    return in_maps


def kernel(func_and_arg: np.ndarray, cooccurrences: np.ndarray) -> np.ndarray:
    assert func_and_arg.shape == (4, 1024, 2 * D)
    assert cooccurrences.shape == (D, D, D)

    in_maps = _prep_in_maps(func_and_arg, cooccurrences)
    nc = _get_nc()
    res = run_bass_kernel_spmd(nc, in_maps, core_ids=list(range(N_CORES)))

    # out_t per core: [z=128, t=512] fp16 -> [t, z]; concat -> [4096, 128]
    outs = [res.results[c]["out_t"].T for c in range(N_CORES)]
    out = np.concatenate(outs, axis=0).reshape(4, 1024, D).astype(np.float32)
    return out
